# revision 7
# baseline (speedup 1.0000x reference)
"""Trainium2 Bass kernel for nn_AttentionModel_47983374631276.

SDPA attention: B=2, H=16, S=2048, D=128, fp8-representable q/k/v with
per-tensor dequant scales (qs, ks, vs).

Sharding: batch*heads = 32 pairs -> 4 heads per core across 8 cores.
Each core runs its full S x S attention locally; no cross-core comm.

Per-head device algorithm (all matmuls bf16 -- lossless for fp8 values):
  1. matmul1: S^T[k, q] = (K^T slice).T @ Q^T   (stationary=K^T [d,128],
     moving=Q^T [d,512], contraction d=128, PSUM f32)
  2. exp: ScalarE activation Exp over [128, 2048] PSUM chunks with the
     free affine scale = qs*ks/sqrt(D) folded in -> P^T bf16 in SBUF.
     No row-max subtraction: |logit| <= ~15, exp stays in fp32 range.
  3. matmul2: out_ext[q, 129] = sum_k P^T[k,q].T @ [V | 1]  (the ones
     column yields the softmax denominator for free)
  4. evac: out[q, :128] * vs / out[q, 128]  on VectorE, DMA to DRAM.
"""

import math
import os

import numpy as np
import ml_dtypes

import concourse.bacc as bacc
import concourse.bass as bass
import concourse.tile as tile
import concourse.mybir as mybir
from concourse.bass_utils import run_bass_kernel_spmd

N_CORES = 8
HEADS_PER_CORE = 4
S = 2048
D = 128
P = 128          # partitions
KT = S // P      # 16 k tiles per head
QQ = 4           # q chunks of 512 for matmul1
QW = S // QQ     # 512

BF16 = mybir.dt.bfloat16
F32 = mybir.dt.float32

# Stash of the most recent run results (exec_time_ns etc.) for test harnesses.
LAST_RESULTS = None
LAST_NC = None


def _build_program(c_scale: float, vs_val: float):
    nc = bacc.Bacc()

    qT_d = nc.dram_tensor("qT", [HEADS_PER_CORE, P, S], BF16, kind="ExternalInput")
    kT_d = nc.dram_tensor("kT", [HEADS_PER_CORE, P, S], BF16, kind="ExternalInput")
    v_d = nc.dram_tensor("v", [HEADS_PER_CORE, S, D], BF16, kind="ExternalInput")
    out_d = nc.dram_tensor("out", [HEADS_PER_CORE, S, D], F32, kind="ExternalOutput")

    with tile.TileContext(nc) as tc:
        with (
            tc.tile_pool(name="io", bufs=2) as io_pool,
            tc.tile_pool(name="ptp", bufs=2) as pt_pool,
            tc.tile_pool(name="outp", bufs=4) as out_pool,
            tc.tile_pool(name="smallp", bufs=4) as small_pool,
            tc.tile_pool(name="ps1p", bufs=1, space="PSUM") as ps1_pool,
            tc.tile_pool(name="ps2p", bufs=3, space="PSUM") as ps2_pool,
        ):
            for h in range(HEADS_PER_CORE):
                qT_sb = io_pool.tile([P, S], BF16, tag="qT")
                nc.sync.dma_start(qT_sb, qT_d[h])
                kT_sb = io_pool.tile([P, S], BF16, tag="kT")
                nc.sync.dma_start(kT_sb, kT_d[h])
                v_sb = io_pool.tile([P, KT, D + 1], BF16, tag="v")
                nc.sync.dma_start(
                    v_sb[:, :, :D], v_d[h].rearrange("(t p) d -> p t d", p=P)
                )
                nc.vector.memset(v_sb[:, :, D : D + 1], 1.0)

                pt_sb = pt_pool.tile([P, KT, S], BF16, tag="pt")

                # Phase 1: logits^T tiles + exp -> P^T
                for kt in range(KT):
                    ps1 = ps1_pool.tile([P, S], F32, tag="ps1")
                    for qq in range(QQ):
                        nc.tensor.matmul(
                            ps1[:, qq * QW : (qq + 1) * QW],
                            lhsT=kT_sb[:, kt * P : (kt + 1) * P],
                            rhs=qT_sb[:, qq * QW : (qq + 1) * QW],
                            start=True,
                            stop=True,
                        )
                    nc.scalar.activation(
                        pt_sb[:, kt, :],
                        ps1[:],
                        mybir.ActivationFunctionType.Exp,
                        scale=c_scale,
                    )

                # Phase 2: out_ext = sum_k P^T.T @ [V | 1], then divide
                for qt in range(KT):
                    ps2 = ps2_pool.tile([P, D + 1], F32, tag="ps2")
                    for kt in range(KT):
                        nc.tensor.matmul(
                            ps2,
                            lhsT=pt_sb[:, kt, qt * P : (qt + 1) * P],
                            rhs=v_sb[:, kt, :],
                            start=(kt == 0),
                            stop=(kt == KT - 1),
                        )
                    recip = small_pool.tile([P, 1], F32, tag="recip")
                    nc.vector.reciprocal(recip, ps2[:, D : D + 1])
                    o_sb = out_pool.tile([P, D], F32, tag="o")
                    nc.vector.tensor_scalar(
                        o_sb,
                        ps2[:, :D],
                        recip,
                        vs_val,
                        mybir.AluOpType.mult,
                        mybir.AluOpType.mult,
                    )
                    nc.sync.dma_start(out_d[h, qt * P : (qt + 1) * P, :], o_sb)

    nc.compile()
    return nc


def kernel(s, q, k, v, qs, ks, vs):
    global LAST_RESULTS, LAST_NC
    q = np.asarray(q, dtype=np.float32)
    k = np.asarray(k, dtype=np.float32)
    v = np.asarray(v, dtype=np.float32)
    qs = np.asarray(qs, dtype=np.float32)
    ks = np.asarray(ks, dtype=np.float32)
    vs = np.asarray(vs, dtype=np.float32)

    B, H, S_, D_ = q.shape
    assert (S_, D_) == (S, D) and B * H == N_CORES * HEADS_PER_CORE

    # fp8-representable values -> bf16 cast is lossless
    qT = np.ascontiguousarray(
        q.reshape(B * H, S, D).transpose(0, 2, 1)
    ).astype(ml_dtypes.bfloat16)
    kT = np.ascontiguousarray(
        k.reshape(B * H, S, D).transpose(0, 2, 1)
    ).astype(ml_dtypes.bfloat16)
    vb = np.ascontiguousarray(v.reshape(B * H, S, D)).astype(ml_dtypes.bfloat16)

    c_scale = float(
        np.float32(qs[0]) * np.float32(ks[0]) * np.float32(1.0 / math.sqrt(D))
    )
    vs_val = float(vs[0])

    nc = _build_program(c_scale, vs_val)
    LAST_NC = nc

    in_maps = []
    for c in range(N_CORES):
        lo, hi = c * HEADS_PER_CORE, (c + 1) * HEADS_PER_CORE
        in_maps.append(
            {
                "qT": np.ascontiguousarray(qT[lo:hi]),
                "kT": np.ascontiguousarray(kT[lo:hi]),
                "v": np.ascontiguousarray(vb[lo:hi]),
            }
        )

    res = run_bass_kernel_spmd(nc, in_maps, core_ids=list(range(N_CORES)))
    LAST_RESULTS = res

    out = np.stack([r["out"] for r in res.results])  # [8, 4, S, D] f32
    return out.reshape(B, H, S, D).astype(np.float32)


# revision 8
# speedup vs baseline: 1.6631x; 1.6631x over previous
"""Trainium2 Bass kernel for nn_AttentionModel_47983374631276.

SDPA attention: B=2, H=16, S=2048, D=128, fp8-representable q/k/v with
per-tensor dequant scales (qs, ks, vs).

Sharding: batch*heads = 32 pairs -> 4 heads per core across 8 cores.
Each core runs its full S x S attention locally; no cross-core comm.

Per-head device algorithm (all matmuls bf16 -- lossless for fp8 values):
  1. matmul1: S^T[k, q] = (K^T slice).T @ Q^T   (stationary=K^T [d,128],
     moving=Q^T [d,512], contraction d=128, PSUM f32)
  2. exp: ScalarE activation Exp over [128, 1536] PSUM chunks with the
     free affine scale = qs*ks/sqrt(D) folded in -> P^T bf16 in SBUF.
     No row-max subtraction: |logit| <= ~15, exp stays in fp32 range.
  3. matmul2: out_ext[q, 129] = sum_k P^T[k,q].T @ [V | 1]  (the ones
     column yields the softmax denominator for free)
  4. evac: out[q, :128] * vs / out[q, 128]  on VectorE, DMA to DRAM.

Software pipelining: phase2 of head h-1 is emitted AFTER phase1 of head
h, so the Tile scheduler keeps ScalarE (the critical engine) fed with
exp work while matmul2 instructions fill TensorE gaps.

PSUM budget (8 banks): psum1 chunks [128,1536] x2 bufs = 6 banks,
ps2 accumulators [128,129] x2 bufs = 2 banks. One accumulation group
per bank (matmul start=True clears has_written for the whole bank).
"""

import math

import numpy as np
import ml_dtypes

import concourse.bacc as bacc
import concourse.bass as bass
import concourse.tile as tile
import concourse.mybir as mybir
from concourse.bass_utils import run_bass_kernel_spmd

N_CORES = 8
HEADS_PER_CORE = 4
S = 2048
D = 128
P = 128            # partitions
KT = S // P        # 16 k tiles per head
QQ = 4             # q chunks of 512 for matmul1
QW = S // QQ       # 512
SLICES = KT * QQ   # 64 matmul1 output slices of 512 per head
CHUNK_SLICES = 3   # exp chunk = 3 x 512 = 1536 elements/partition

BF16 = mybir.dt.bfloat16
F32 = mybir.dt.float32

# Stash of the most recent run results / program for test harnesses.
LAST_RESULTS = None
LAST_NC = None


def _build_program(c_scale: float, vs_val: float):
    nc = bacc.Bacc()

    qT_d = nc.dram_tensor("qT", [HEADS_PER_CORE, P, S], BF16, kind="ExternalInput")
    kT_d = nc.dram_tensor("kT", [HEADS_PER_CORE, P, S], BF16, kind="ExternalInput")
    v_d = nc.dram_tensor("v", [HEADS_PER_CORE, S, D], BF16, kind="ExternalInput")
    out_d = nc.dram_tensor("out", [HEADS_PER_CORE, S, D], F32, kind="ExternalOutput")

    with tile.TileContext(nc) as tc:
        with (
            tc.tile_pool(name="io", bufs=2) as io_pool,
            tc.tile_pool(name="ptp", bufs=2) as pt_pool,
            tc.tile_pool(name="outp", bufs=4) as out_pool,
            tc.tile_pool(name="smallp", bufs=4) as small_pool,
            tc.tile_pool(name="ps1p", bufs=2, space="PSUM") as ps1_pool,
            tc.tile_pool(name="ps2p", bufs=2, space="PSUM") as ps2_pool,
        ):

            def emit_load(h):
                qT_sb = io_pool.tile([P, S], BF16, tag="qT")
                nc.sync.dma_start(qT_sb, qT_d[h])
                kT_sb = io_pool.tile([P, S], BF16, tag="kT")
                nc.sync.dma_start(kT_sb, kT_d[h])
                v_sb = io_pool.tile([P, KT, D + 1], BF16, tag="v")
                nc.sync.dma_start(
                    v_sb[:, :, :D], v_d[h].rearrange("(t p) d -> p t d", p=P)
                )
                nc.vector.memset(v_sb[:, :, D : D + 1], 1.0)
                return qT_sb, kT_sb, v_sb

            def emit_phase1(qT_sb, kT_sb):
                # P^T flat layout: element (k_in_tile, kt*S + q)
                pt_sb = pt_pool.tile([P, KT * S], BF16, tag="pt")
                s = 0
                while s < SLICES:
                    n = min(CHUNK_SLICES, SLICES - s)
                    ps1 = ps1_pool.tile([P, CHUNK_SLICES * QW], F32, tag="ps1")
                    for j in range(n):
                        kt, qq = divmod(s + j, QQ)
                        nc.tensor.matmul(
                            ps1[:, j * QW : (j + 1) * QW],
                            lhsT=kT_sb[:, kt * P : (kt + 1) * P],
                            rhs=qT_sb[:, qq * QW : (qq + 1) * QW],
                            start=True,
                            stop=True,
                        )
                    nc.scalar.activation(
                        pt_sb[:, s * QW : (s + n) * QW],
                        ps1[:, : n * QW],
                        mybir.ActivationFunctionType.Exp,
                        scale=c_scale,
                    )
                    s += n
                return pt_sb

            def emit_phase2(h, pt_sb, v_sb):
                for qt in range(KT):
                    ps2 = ps2_pool.tile([P, D + 1], F32, tag="ps2")
                    for kt in range(KT):
                        off = kt * S + qt * P
                        nc.tensor.matmul(
                            ps2,
                            lhsT=pt_sb[:, off : off + P],
                            rhs=v_sb[:, kt, :],
                            start=(kt == 0),
                            stop=(kt == KT - 1),
                        )
                    recip = small_pool.tile([P, 1], F32, tag="recip")
                    nc.vector.reciprocal(recip, ps2[:, D : D + 1])
                    o_sb = out_pool.tile([P, D], F32, tag="o")
                    nc.vector.tensor_scalar(
                        o_sb,
                        ps2[:, :D],
                        recip,
                        vs_val,
                        mybir.AluOpType.mult,
                        mybir.AluOpType.mult,
                    )
                    nc.sync.dma_start(out_d[h, qt * P : (qt + 1) * P, :], o_sb)

            prev = None
            for h in range(HEADS_PER_CORE):
                qT_sb, kT_sb, v_sb = emit_load(h)
                pt_sb = emit_phase1(qT_sb, kT_sb)
                if prev is not None:
                    emit_phase2(*prev)
                prev = (h, pt_sb, v_sb)
            emit_phase2(*prev)

    nc.compile()
    return nc


def kernel(s, q, k, v, qs, ks, vs):
    global LAST_RESULTS, LAST_NC
    q = np.asarray(q, dtype=np.float32)
    k = np.asarray(k, dtype=np.float32)
    v = np.asarray(v, dtype=np.float32)
    qs = np.asarray(qs, dtype=np.float32)
    ks = np.asarray(ks, dtype=np.float32)
    vs = np.asarray(vs, dtype=np.float32)

    B, H, S_, D_ = q.shape
    assert (S_, D_) == (S, D) and B * H == N_CORES * HEADS_PER_CORE

    # fp8-representable values -> bf16 cast is lossless
    qT = np.ascontiguousarray(
        q.reshape(B * H, S, D).transpose(0, 2, 1)
    ).astype(ml_dtypes.bfloat16)
    kT = np.ascontiguousarray(
        k.reshape(B * H, S, D).transpose(0, 2, 1)
    ).astype(ml_dtypes.bfloat16)
    vb = np.ascontiguousarray(v.reshape(B * H, S, D)).astype(ml_dtypes.bfloat16)

    c_scale = float(
        np.float32(qs[0]) * np.float32(ks[0]) * np.float32(1.0 / math.sqrt(D))
    )
    vs_val = float(vs[0])

    nc = _build_program(c_scale, vs_val)
    LAST_NC = nc

    in_maps = []
    for c in range(N_CORES):
        lo, hi = c * HEADS_PER_CORE, (c + 1) * HEADS_PER_CORE
        in_maps.append(
            {
                "qT": np.ascontiguousarray(qT[lo:hi]),
                "kT": np.ascontiguousarray(kT[lo:hi]),
                "v": np.ascontiguousarray(vb[lo:hi]),
            }
        )

    res = run_bass_kernel_spmd(nc, in_maps, core_ids=list(range(N_CORES)))
    LAST_RESULTS = res

    out = np.stack([r["out"] for r in res.results])  # [8, 4, S, D] f32
    return out.reshape(B, H, S, D).astype(np.float32)


# revision 11
# speedup vs baseline: 1.6746x; 1.0069x over previous
"""Trainium2 Bass kernel for nn_AttentionModel_47983374631276.

SDPA attention: B=2, H=16, S=2048, D=128, fp8-representable q/k/v with
per-tensor dequant scales (qs, ks, vs).

Sharding: batch*heads = 32 pairs -> 4 heads per core across 8 cores.
Each core runs its full S x S attention locally; no cross-core comm.

Per-head device algorithm (all matmuls bf16 -- lossless for fp8 values):
  1. matmul1: S^T[k, q] = (K^T slice).T @ Q^T   (stationary=K^T [d,128],
     moving=Q^T [d,512], contraction d=128, PSUM f32)
  2. exp: ScalarE activation Exp over [128, 1536] PSUM chunks with the
     free affine scale = qs*ks/sqrt(D) folded in -> P^T bf16 in SBUF.
     No row-max subtraction: |logit| <= ~15, exp stays in fp32 range.
  3. matmul2: out_ext[q, 129] = sum_k P^T[k,q].T @ [V | 1]  (the ones
     column yields the softmax denominator for free)
  4. evac: out[q, :128] * vs / out[q, 128]  on VectorE, DMA to DRAM.

Software pipelining: phase2 of head h-1 is emitted AFTER phase1 of head
h, so the Tile scheduler keeps ScalarE (the critical engine) fed with
exp work while matmul2 instructions fill TensorE gaps.

PSUM budget (8 banks): psum1 chunks [128,1536] x2 bufs = 6 banks,
ps2 accumulators [128,129] x2 bufs = 2 banks. One accumulation group
per bank (matmul start=True clears has_written for the whole bank).
"""

import math

import numpy as np
import ml_dtypes

import concourse.bacc as bacc
import concourse.bass as bass
import concourse.tile as tile
import concourse.mybir as mybir
from concourse.bass_utils import run_bass_kernel_spmd

N_CORES = 8
HEADS_PER_CORE = 4
S = 2048
D = 128
P = 128            # partitions
KT = S // P        # 16 k tiles per head
QQ = 4             # q chunks of 512 for matmul1
QW = S // QQ       # 512
SLICES = KT * QQ   # 64 matmul1 output slices of 512 per head
CHUNK_SLICES = 3   # exp chunk = 3 x 512 = 1536 elements/partition

BF16 = mybir.dt.bfloat16
F32 = mybir.dt.float32

# Stash of the most recent run results / program for test harnesses.
LAST_RESULTS = None
LAST_NC = None


def _build_program(c_scale: float, vs_val: float):
    nc = bacc.Bacc()

    qT_d = nc.dram_tensor("qT", [HEADS_PER_CORE, P, S], BF16, kind="ExternalInput")
    kT_d = nc.dram_tensor("kT", [HEADS_PER_CORE, P, S], BF16, kind="ExternalInput")
    v_d = nc.dram_tensor("v", [HEADS_PER_CORE, S, D], BF16, kind="ExternalInput")
    out_d = nc.dram_tensor("out", [HEADS_PER_CORE, S, D], F32, kind="ExternalOutput")

    with tile.TileContext(nc) as tc:
        with (
            tc.tile_pool(name="io", bufs=2) as io_pool,
            tc.tile_pool(name="ptp", bufs=2) as pt_pool,
            tc.tile_pool(name="outp", bufs=4) as out_pool,
            tc.tile_pool(name="smallp", bufs=4) as small_pool,
            tc.tile_pool(name="ps1p", bufs=2, space="PSUM") as ps1_pool,
            tc.tile_pool(name="ps2p", bufs=2, space="PSUM") as ps2_pool,
        ):

            # Priority bands: the Tile list-scheduler prefers smaller
            # bass_priority among ready instructions. Keep all loads +
            # phase1 (matmul1 + exp — the ACT-critical chain) in a low
            # band so leftover phase2 matmuls never starve the next
            # head's phase1 on the in-order PE stream.
            P1_BAND = 0
            P2_BAND = 10_000_000
            HEAD_STRIDE = 100_000

            def emit_load(h):
                tc.cur_priority = P1_BAND + h * HEAD_STRIDE
                qT_sb = io_pool.tile([P, S], BF16, tag="qT")
                nc.sync.dma_start(qT_sb, qT_d[h])
                kT_sb = io_pool.tile([P, S], BF16, tag="kT")
                nc.sync.dma_start(kT_sb, kT_d[h])
                v_sb = io_pool.tile([P, KT, D + 1], BF16, tag="v")
                nc.sync.dma_start(
                    v_sb[:, :, :D], v_d[h].rearrange("(t p) d -> p t d", p=P)
                )
                nc.vector.memset(v_sb[:, :, D : D + 1], 1.0)
                return qT_sb, kT_sb, v_sb

            def emit_phase1(h, qT_sb, kT_sb):
                tc.cur_priority = P1_BAND + h * HEAD_STRIDE + 1000
                # P^T flat layout: element (k_in_tile, kt*S + q)
                pt_sb = pt_pool.tile([P, KT * S], BF16, tag="pt")
                # First chunk is 1 slice: its exp depends on a single
                # matmul, so each head's ACT stream restarts with a
                # minimal PE dependency right at the head boundary.
                s = 0
                first = True
                while s < SLICES:
                    n = 1 if first else min(CHUNK_SLICES, SLICES - s)
                    first = False
                    ps1 = ps1_pool.tile([P, CHUNK_SLICES * QW], F32, tag="ps1")
                    for j in range(n):
                        kt, qq = divmod(s + j, QQ)
                        nc.tensor.matmul(
                            ps1[:, j * QW : (j + 1) * QW],
                            lhsT=kT_sb[:, kt * P : (kt + 1) * P],
                            rhs=qT_sb[:, qq * QW : (qq + 1) * QW],
                            start=True,
                            stop=True,
                        )
                    nc.scalar.activation(
                        pt_sb[:, s * QW : (s + n) * QW],
                        ps1[:, : n * QW],
                        mybir.ActivationFunctionType.Exp,
                        scale=c_scale,
                    )
                    s += n
                return pt_sb

            def emit_phase2(h, pt_sb, v_sb):
                tc.cur_priority = P2_BAND + h * HEAD_STRIDE
                for qt in range(KT):
                    ps2 = ps2_pool.tile([P, D + 1], F32, tag="ps2")
                    for kt in range(KT):
                        off = kt * S + qt * P
                        nc.tensor.matmul(
                            ps2,
                            lhsT=pt_sb[:, off : off + P],
                            rhs=v_sb[:, kt, :],
                            start=(kt == 0),
                            stop=(kt == KT - 1),
                        )
                    recip = small_pool.tile([P, 1], F32, tag="recip")
                    nc.vector.reciprocal(recip, ps2[:, D : D + 1])
                    o_sb = out_pool.tile([P, D], F32, tag="o")
                    nc.vector.tensor_scalar(
                        o_sb,
                        ps2[:, :D],
                        recip,
                        vs_val,
                        mybir.AluOpType.mult,
                        mybir.AluOpType.mult,
                    )
                    nc.sync.dma_start(out_d[h, qt * P : (qt + 1) * P, :], o_sb)

            prev = None
            for h in range(HEADS_PER_CORE):
                qT_sb, kT_sb, v_sb = emit_load(h)
                pt_sb = emit_phase1(h, qT_sb, kT_sb)
                if prev is not None:
                    emit_phase2(*prev)
                prev = (h, pt_sb, v_sb)
            emit_phase2(*prev)

    nc.compile()
    return nc


def kernel(s, q, k, v, qs, ks, vs):
    global LAST_RESULTS, LAST_NC
    q = np.asarray(q, dtype=np.float32)
    k = np.asarray(k, dtype=np.float32)
    v = np.asarray(v, dtype=np.float32)
    qs = np.asarray(qs, dtype=np.float32)
    ks = np.asarray(ks, dtype=np.float32)
    vs = np.asarray(vs, dtype=np.float32)

    B, H, S_, D_ = q.shape
    assert (S_, D_) == (S, D) and B * H == N_CORES * HEADS_PER_CORE

    # fp8-representable values -> bf16 cast is lossless
    qT = np.ascontiguousarray(
        q.reshape(B * H, S, D).transpose(0, 2, 1)
    ).astype(ml_dtypes.bfloat16)
    kT = np.ascontiguousarray(
        k.reshape(B * H, S, D).transpose(0, 2, 1)
    ).astype(ml_dtypes.bfloat16)
    vb = np.ascontiguousarray(v.reshape(B * H, S, D)).astype(ml_dtypes.bfloat16)

    c_scale = float(
        np.float32(qs[0]) * np.float32(ks[0]) * np.float32(1.0 / math.sqrt(D))
    )
    vs_val = float(vs[0])

    nc = _build_program(c_scale, vs_val)
    LAST_NC = nc

    in_maps = []
    for c in range(N_CORES):
        lo, hi = c * HEADS_PER_CORE, (c + 1) * HEADS_PER_CORE
        in_maps.append(
            {
                "qT": np.ascontiguousarray(qT[lo:hi]),
                "kT": np.ascontiguousarray(kT[lo:hi]),
                "v": np.ascontiguousarray(vb[lo:hi]),
            }
        )

    res = run_bass_kernel_spmd(nc, in_maps, core_ids=list(range(N_CORES)))
    LAST_RESULTS = res

    out = np.stack([r["out"] for r in res.results])  # [8, 4, S, D] f32
    return out.reshape(B, H, S, D).astype(np.float32)


# revision 15
# speedup vs baseline: 1.7255x; 1.0304x over previous
"""Trainium2 Bass kernel for nn_AttentionModel_47983374631276.

SDPA attention: B=2, H=16, S=2048, D=128, fp8-representable q/k/v with
per-tensor dequant scales (qs, ks, vs).

Sharding: batch*heads = 32 pairs -> 4 heads per core across 8 cores.
Each core runs its full S x S attention locally; no cross-core comm.

Per-head device algorithm (all matmuls bf16 -- lossless for fp8 values):
  1. matmul1: S^T[k, q] = (K^T slice).T @ Q^T   (stationary=K^T [d,128],
     moving=Q^T [d,512], contraction d=128, PSUM f32)
  2. exp: ScalarE activation Exp over [128, 1536] PSUM chunks with the
     free affine scale = qs*ks/sqrt(D) folded in -> P^T bf16 in SBUF.
     No row-max subtraction: |logit| <= ~15, exp stays in fp32 range.
  3. matmul2: out_ext[q, 129] = sum_k P^T[k,q].T @ [V | 1]  (the ones
     column yields the softmax denominator for free)
  4. evac: out[q, :128] * vs / out[q, 128]  on VectorE, DMA to DRAM.

Software pipelining: phase2 of head h-1 is emitted AFTER phase1 of head
h, so the Tile scheduler keeps ScalarE (the critical engine) fed with
exp work while matmul2 instructions fill TensorE gaps.

PSUM budget (8 banks): psum1 chunks [128,1536] x2 bufs = 6 banks,
ps2 accumulators [128,129] x2 bufs = 2 banks. One accumulation group
per bank (matmul start=True clears has_written for the whole bank).
"""

import math

import numpy as np
import ml_dtypes

import concourse.bacc as bacc
import concourse.bass as bass
import concourse.tile as tile
import concourse.mybir as mybir
from concourse.bass_utils import run_bass_kernel_spmd

N_CORES = 8
HEADS_PER_CORE = 4
S = 2048
D = 128
P = 128            # partitions
KT = S // P        # 16 k tiles per head
QQ = 4             # q chunks of 512 for matmul1
QW = S // QQ       # 512
SLICES = KT * QQ   # 64 matmul1 output slices of 512 per head
CHUNK_SLICES = 3   # exp chunk = 3 x 512 = 1536 elements/partition

BF16 = mybir.dt.bfloat16
F32 = mybir.dt.float32

# Stash of the most recent run results / program for test harnesses.
LAST_RESULTS = None
LAST_NC = None


def _build_program(c_scale: float, vs_val: float):
    nc = bacc.Bacc()

    qT_d = nc.dram_tensor("qT", [HEADS_PER_CORE, P, S], BF16, kind="ExternalInput")
    kT_d = nc.dram_tensor("kT", [HEADS_PER_CORE, P, S], BF16, kind="ExternalInput")
    v_d = nc.dram_tensor("v", [HEADS_PER_CORE, S, D], BF16, kind="ExternalInput")
    out_d = nc.dram_tensor("out", [HEADS_PER_CORE, S, D], F32, kind="ExternalOutput")

    with tile.TileContext(nc) as tc:
        with (
            tc.tile_pool(name="io", bufs=2) as io_pool,
            tc.tile_pool(name="ptp", bufs=2 * QQ) as pt_pool,
            tc.tile_pool(name="outp", bufs=4) as out_pool,
            tc.tile_pool(name="smallp", bufs=4) as small_pool,
            tc.tile_pool(name="ps1p", bufs=2, space="PSUM") as ps1_pool,
            tc.tile_pool(name="ps2p", bufs=2, space="PSUM") as ps2_pool,
        ):

            # Priority bands: the Tile list-scheduler prefers smaller
            # bass_priority among ready instructions. Keep all loads +
            # phase1 (matmul1 + exp — the ACT-critical chain) in a low
            # band so leftover phase2 matmuls never starve the next
            # head's phase1 on the in-order PE stream.
            P1_BAND = 0
            P2_BAND = 10_000_000
            HEAD_STRIDE = 100_000

            def emit_load(h):
                tc.cur_priority = P1_BAND + h * HEAD_STRIDE
                qT_sb = io_pool.tile([P, S], BF16, tag="qT")
                nc.sync.dma_start(qT_sb, qT_d[h])
                kT_sb = io_pool.tile([P, S], BF16, tag="kT")
                nc.sync.dma_start(kT_sb, kT_d[h])
                v_sb = io_pool.tile([P, KT, D + 1], BF16, tag="v")
                nc.sync.dma_start(
                    v_sb[:, :, :D], v_d[h].rearrange("(t p) d -> p t d", p=P)
                )
                nc.vector.memset(v_sb[:, :, D : D + 1], 1.0)
                return qT_sb, kT_sb, v_sb

            def emit_phase1(h, qT_sb, kT_sb):
                tc.cur_priority = P1_BAND + h * HEAD_STRIDE + 1000
                # P^T stored as 4 q-quarter tiles [P, kt, 512] so the
                # slot WAR (bufs=8 = 2 heads in flight) couples each exp
                # chunk only to the 4 matmul2 groups reading the same
                # quarter two heads earlier, not to a whole phase2.
                quarters = []
                for qq in range(QQ):
                    ptq = pt_pool.tile([P, KT, QW], BF16, tag="ptq")
                    quarters.append(ptq)
                    # 16 kt slices per quarter, chunked [1,3,3,3,3,3]:
                    # the leading 1-slice chunk restarts the ACT stream
                    # with a minimal PE dependency at quarter entry.
                    kt0 = 0
                    for n in (1, 3, 3, 3, 3, 3):
                        ps1 = ps1_pool.tile([P, CHUNK_SLICES, QW], F32, tag="ps1")
                        for j in range(n):
                            kt = kt0 + j
                            nc.tensor.matmul(
                                ps1[:, j, :],
                                lhsT=kT_sb[:, kt * P : (kt + 1) * P],
                                rhs=qT_sb[:, qq * QW : (qq + 1) * QW],
                                start=True,
                                stop=True,
                            )
                        nc.scalar.activation(
                            ptq[:, kt0 : kt0 + n, :],
                            ps1[:, :n, :],
                            mybir.ActivationFunctionType.Exp,
                            scale=c_scale,
                        )
                        kt0 += n
                return quarters

            def emit_phase2(h, quarters, v_sb):
                tc.cur_priority = P2_BAND + h * HEAD_STRIDE
                for qt in range(KT):
                    ptq = quarters[qt // QQ]
                    col = (qt % QQ) * P
                    ps2 = ps2_pool.tile([P, D + 1], F32, tag="ps2")
                    for kt in range(KT):
                        nc.tensor.matmul(
                            ps2,
                            lhsT=ptq[:, kt, col : col + P],
                            rhs=v_sb[:, kt, :],
                            start=(kt == 0),
                            stop=(kt == KT - 1),
                        )
                    recip = small_pool.tile([P, 1], F32, tag="recip")
                    nc.vector.reciprocal(recip, ps2[:, D : D + 1])
                    o_sb = out_pool.tile([P, D], F32, tag="o")
                    nc.vector.tensor_scalar(
                        o_sb,
                        ps2[:, :D],
                        recip,
                        vs_val,
                        mybir.AluOpType.mult,
                        mybir.AluOpType.mult,
                    )
                    nc.sync.dma_start(out_d[h, qt * P : (qt + 1) * P, :], o_sb)

            prev = None
            for h in range(HEADS_PER_CORE):
                qT_sb, kT_sb, v_sb = emit_load(h)
                quarters = emit_phase1(h, qT_sb, kT_sb)
                if prev is not None:
                    emit_phase2(*prev)
                prev = (h, quarters, v_sb)
            emit_phase2(*prev)

    nc.compile()
    return nc


def kernel(s, q, k, v, qs, ks, vs):
    global LAST_RESULTS, LAST_NC
    q = np.asarray(q, dtype=np.float32)
    k = np.asarray(k, dtype=np.float32)
    v = np.asarray(v, dtype=np.float32)
    qs = np.asarray(qs, dtype=np.float32)
    ks = np.asarray(ks, dtype=np.float32)
    vs = np.asarray(vs, dtype=np.float32)

    B, H, S_, D_ = q.shape
    assert (S_, D_) == (S, D) and B * H == N_CORES * HEADS_PER_CORE

    # fp8-representable values -> bf16 cast is lossless
    qT = np.ascontiguousarray(
        q.reshape(B * H, S, D).transpose(0, 2, 1)
    ).astype(ml_dtypes.bfloat16)
    kT = np.ascontiguousarray(
        k.reshape(B * H, S, D).transpose(0, 2, 1)
    ).astype(ml_dtypes.bfloat16)
    vb = np.ascontiguousarray(v.reshape(B * H, S, D)).astype(ml_dtypes.bfloat16)

    c_scale = float(
        np.float32(qs[0]) * np.float32(ks[0]) * np.float32(1.0 / math.sqrt(D))
    )
    vs_val = float(vs[0])

    nc = _build_program(c_scale, vs_val)
    LAST_NC = nc

    in_maps = []
    for c in range(N_CORES):
        lo, hi = c * HEADS_PER_CORE, (c + 1) * HEADS_PER_CORE
        in_maps.append(
            {
                "qT": np.ascontiguousarray(qT[lo:hi]),
                "kT": np.ascontiguousarray(kT[lo:hi]),
                "v": np.ascontiguousarray(vb[lo:hi]),
            }
        )

    res = run_bass_kernel_spmd(nc, in_maps, core_ids=list(range(N_CORES)))
    LAST_RESULTS = res

    out = np.stack([r["out"] for r in res.results])  # [8, 4, S, D] f32
    return out.reshape(B, H, S, D).astype(np.float32)


# revision 20
# speedup vs baseline: 1.8255x; 1.0579x over previous
"""Trainium2 Bass kernel for nn_AttentionModel_47983374631276.

SDPA attention: B=2, H=16, S=2048, D=128, fp8-representable q/k/v with
per-tensor dequant scales (qs, ks, vs).

Sharding: batch*heads = 32 pairs -> 4 heads per core across 8 cores.
Each core runs its full S x S attention locally; no cross-core comm.

Per-head device algorithm (all matmuls bf16 -- lossless for fp8 values):
  1. matmul1: S^T[k, q] = (K^T slice).T @ Q^T   (stationary=K^T [d,128],
     moving=Q^T [d,512], contraction d=128, PSUM f32)
  2. exp: ScalarE activation Exp over [128, 1536] PSUM chunks with the
     free affine scale = qs*ks/sqrt(D) folded in -> P^T bf16 in SBUF.
     No row-max subtraction: |logit| <= ~15, exp stays in fp32 range.
  3. matmul2: out_ext[q, 129] = sum_k P^T[k,q].T @ [V | 1]  (the ones
     column yields the softmax denominator for free)
  4. evac: out[q, :128] * vs / out[q, 128]  on VectorE, DMA to DRAM.

Software pipelining: phase2 of head h-1 is emitted AFTER phase1 of head
h, so the Tile scheduler keeps ScalarE (the critical engine) fed with
exp work while matmul2 instructions fill TensorE gaps.

PSUM budget (8 banks): psum1 chunks [128,1536] x2 bufs = 6 banks,
ps2 accumulators [128,129] x2 bufs = 2 banks. One accumulation group
per bank (matmul start=True clears has_written for the whole bank).
"""

import math

import numpy as np
import ml_dtypes

import concourse.bacc as bacc
import concourse.bass as bass
import concourse.tile as tile
import concourse.mybir as mybir
from concourse.bass_utils import run_bass_kernel_spmd

N_CORES = 8
HEADS_PER_CORE = 4
S = 2048
D = 128
P = 128            # partitions
KT = S // P        # 16 k tiles per head
QQ = 4             # q chunks of 512 for matmul1
QW = S // QQ       # 512
SLICES = KT * QQ   # 64 matmul1 output slices of 512 per head
CHUNK_SLICES = 3   # exp chunk = 3 x 512 = 1536 elements/partition

BF16 = mybir.dt.bfloat16
F32 = mybir.dt.float32

# Stash of the most recent run results / program for test harnesses.
LAST_RESULTS = None
LAST_NC = None


def _build_program(c_scale: float, vs_val: float):
    nc = bacc.Bacc()

    qT_d = nc.dram_tensor("qT", [HEADS_PER_CORE, P, S], BF16, kind="ExternalInput")
    kT_d = nc.dram_tensor("kT", [HEADS_PER_CORE, P, S], BF16, kind="ExternalInput")
    v_d = nc.dram_tensor("v", [HEADS_PER_CORE, S, D], BF16, kind="ExternalInput")
    out_d = nc.dram_tensor("out", [HEADS_PER_CORE, S, D], F32, kind="ExternalOutput")

    with tile.TileContext(nc) as tc:
        with (
            tc.tile_pool(name="io", bufs=2) as io_pool,
            tc.tile_pool(name="ptp", bufs=4) as pt_pool,
            tc.tile_pool(name="outp", bufs=4) as out_pool,
            tc.tile_pool(name="smallp", bufs=4) as small_pool,
            tc.tile_pool(name="ps1p", bufs=2, space="PSUM") as ps1_pool,
            tc.tile_pool(name="ps2p", bufs=2, space="PSUM") as ps2_pool,
        ):

            # Priority bands: the Tile list-scheduler prefers smaller
            # bass_priority among ready instructions. Keep all loads +
            # phase1 (matmul1 + exp — the ACT-critical chain) in a low
            # band so leftover phase2 matmuls never starve the next
            # head's phase1 on the in-order PE stream.
            P1_BAND = 0
            P2_BAND = 10_000_000
            HEAD_STRIDE = 100_000

            def emit_load(h):
                tc.cur_priority = P1_BAND + h * HEAD_STRIDE
                # Split the K^T/Q^T loads into column blocks so the first
                # exp chunk's matmuls depend on ~0.7us of DMA, not 2.8us
                # (Tile subtile deps track per-range coverage). v is only
                # needed by phase2, a full head later.
                kT_sb = io_pool.tile([P, S], BF16, tag="kT")
                qT_sb = io_pool.tile([P, S], BF16, tag="qT")
                for b in range(QQ):
                    sl = slice(b * QW, (b + 1) * QW)
                    nc.sync.dma_start(kT_sb[:, sl], kT_d[h, :, sl])
                    nc.sync.dma_start(qT_sb[:, sl], qT_d[h, :, sl])
                v_sb = io_pool.tile([P, KT, D + 1], BF16, tag="v")
                nc.sync.dma_start(
                    v_sb[:, :, :D], v_d[h].rearrange("(t p) d -> p t d", p=P)
                )
                nc.vector.memset(v_sb[:, :, D : D + 1], 1.0)
                return qT_sb, kT_sb, v_sb

            def emit_phase1(h, qT_sb, kT_sb):
                tc.cur_priority = P1_BAND + h * HEAD_STRIDE + 1000
                # P^T stored as 2 q-half tiles [P, kt_slice...] so the
                # slot WAR (bufs=4 = 2 heads in flight) couples each exp
                # chunk only to the 8 matmul2 groups reading the same
                # half two heads earlier, not to a whole phase2.
                # Each half covers q slices (qq, qq+1): 32 [128,512]
                # slices, chunked [2, 3x10]: the leading 2-slice chunk
                # restarts the ACT stream with a small PE dependency.
                halves = []
                for hh in range(2):
                    pth = pt_pool.tile([P, 2 * KT, QW], BF16, tag="pth")
                    halves.append(pth)
                    s0 = 0  # slice index within the half: s = qq_loc*KT + kt
                    for n in (2, 3, 3, 3, 3, 3, 3, 3, 3, 3, 3):
                        ps1 = ps1_pool.tile([P, CHUNK_SLICES, QW], F32, tag="ps1")
                        for j in range(n):
                            qq_loc, kt = divmod(s0 + j, KT)
                            nc.tensor.matmul(
                                ps1[:, j, :],
                                lhsT=kT_sb[:, kt * P : (kt + 1) * P],
                                rhs=qT_sb[
                                    :,
                                    (2 * hh + qq_loc) * QW : (2 * hh + qq_loc + 1) * QW,
                                ],
                                start=True,
                                stop=True,
                            )
                        nc.scalar.activation(
                            pth[:, s0 : s0 + n, :],
                            ps1[:, :n, :],
                            mybir.ActivationFunctionType.Exp,
                            scale=c_scale,
                        )
                        s0 += n
                return halves

            def emit_phase2(h, halves, v_sb):
                tc.cur_priority = P2_BAND + h * HEAD_STRIDE
                for qt in range(KT):
                    pth = halves[qt // (2 * QQ)]
                    qq_loc, qcol = divmod(qt % (2 * QQ), QQ)
                    ps2 = ps2_pool.tile([P, D + 1], F32, tag="ps2")
                    for kt in range(KT):
                        nc.tensor.matmul(
                            ps2,
                            lhsT=pth[:, qq_loc * KT + kt, qcol * P : (qcol + 1) * P],
                            rhs=v_sb[:, kt, :],
                            start=(kt == 0),
                            stop=(kt == KT - 1),
                        )
                    recip = small_pool.tile([P, 1], F32, tag="recip")
                    nc.vector.reciprocal(recip, ps2[:, D : D + 1])
                    o_sb = out_pool.tile([P, D], F32, tag="o")
                    nc.vector.tensor_scalar(
                        o_sb,
                        ps2[:, :D],
                        recip,
                        vs_val,
                        mybir.AluOpType.mult,
                        mybir.AluOpType.mult,
                    )
                    nc.sync.dma_start(out_d[h, qt * P : (qt + 1) * P, :], o_sb)

            prev = None
            for h in range(HEADS_PER_CORE):
                qT_sb, kT_sb, v_sb = emit_load(h)
                halves = emit_phase1(h, qT_sb, kT_sb)
                if prev is not None:
                    emit_phase2(*prev)
                prev = (h, halves, v_sb)
            emit_phase2(*prev)

    nc.compile()
    return nc


def kernel(s, q, k, v, qs, ks, vs):
    global LAST_RESULTS, LAST_NC
    q = np.asarray(q, dtype=np.float32)
    k = np.asarray(k, dtype=np.float32)
    v = np.asarray(v, dtype=np.float32)
    qs = np.asarray(qs, dtype=np.float32)
    ks = np.asarray(ks, dtype=np.float32)
    vs = np.asarray(vs, dtype=np.float32)

    B, H, S_, D_ = q.shape
    assert (S_, D_) == (S, D) and B * H == N_CORES * HEADS_PER_CORE

    # fp8-representable values -> bf16 cast is lossless
    qT = np.ascontiguousarray(
        q.reshape(B * H, S, D).transpose(0, 2, 1)
    ).astype(ml_dtypes.bfloat16)
    kT = np.ascontiguousarray(
        k.reshape(B * H, S, D).transpose(0, 2, 1)
    ).astype(ml_dtypes.bfloat16)
    vb = np.ascontiguousarray(v.reshape(B * H, S, D)).astype(ml_dtypes.bfloat16)

    c_scale = float(
        np.float32(qs[0]) * np.float32(ks[0]) * np.float32(1.0 / math.sqrt(D))
    )
    vs_val = float(vs[0])

    nc = _build_program(c_scale, vs_val)
    LAST_NC = nc

    in_maps = []
    for c in range(N_CORES):
        lo, hi = c * HEADS_PER_CORE, (c + 1) * HEADS_PER_CORE
        in_maps.append(
            {
                "qT": np.ascontiguousarray(qT[lo:hi]),
                "kT": np.ascontiguousarray(kT[lo:hi]),
                "v": np.ascontiguousarray(vb[lo:hi]),
            }
        )

    res = run_bass_kernel_spmd(nc, in_maps, core_ids=list(range(N_CORES)))
    LAST_RESULTS = res

    out = np.stack([r["out"] for r in res.results])  # [8, 4, S, D] f32
    return out.reshape(B, H, S, D).astype(np.float32)
